# revision 66
# baseline (speedup 1.0000x reference)
"""GCN message-passing kernel for Trainium2, 8-core SPMD — v4 (fp8, software-
pipelined emission).

Data plan (per core, NS=1024 local rows, JT=64 node tiles, fp8 m):
  * m pre-cast to fp8e4m3 on the HOST (layout/dtype prep only): 8 MB/core per
    rep (~24 us window at ~350 GB/s).
  * Both SpMM layers are fp8 DoubleRow matmuls (two j-tiles contracted per
    pass). Messages carry a power-of-2 gain G=64 folded exactly into the
    degree scales so fp8 stays in the normal range. An optional fp8 residual
    stream for msg2 (_USE_RESIDUAL) buys ~1e-3 rel err at +1 MB of AllGather
    and 2x phase E; it is OFF by default (rel err ~1.2e-2 vs the 2e-2 gate)
    because the serialized HW collective latency dominates the marginal rep
    time (~51 us/rep measured for RS32K+AG1M+AG2M).
  * PSUM accumulators are zeroed by explicit zero-stationary matmul passes;
    all accumulating matmuls use start=False (a start=True write zeroes the
    whole 2KB bank region, which would wipe interleaved 1KB slices).
  * msg1 computed for local nodes only, scaled by G*s_cl (one ReduceScatter
    of the column-degree partials), AllGathered as fp8.
  * Phase D folds alpha = s_cl*s_r into the t1 stationary via a
    partition-broadcast of the alpha row, so relu+quantize run as two batched
    ACT ops over [128,1024] (no per-tile scales, no activation-table thrash).
  * Column-degree partials split ACT (Copy+accum_out) / DVE (reduce) per
    chunk; rd via DoubleRow ones-matmuls on PE.

Scheduling plan: engines execute their instruction streams in order, so late
ops of rep r (msg2 AllGather + readback, phase F elementwise, classifier, out
DMA) are EMITTED inside rep r+1/r+2's streams. This keeps every queue's
emission order consistent with execution order across rep boundaries and lets
rep r+1's m-window overlap rep r's tail (the marginal rep time is what the
harness measures).
  SP/sync : [out dma (r-2)] [m chunks] [cd_part out] [cdl tiles/row in]
            [msg1s readback]
  Pool    : [gathers] [msg2 out+AG2+readback (r-1)] [RS] [msg1q out] [AG1]
            [alpha/beta broadcasts]
  ACT     : [B relus + hT copies] [cd share] [sqrts] [scale8] [D relus]
  DVE     : [cd share] [recips/alpha] [F mul + segmax + out copy (r-1)]
            [t1*alpha]
  PE      : [B transp/GEMM] [rd] [E (r-1)] [classifier (r-1)] [C] [D GEMMs]
"""

import sys

for p in ("/opt/trn_rl_repo",):
    if p not in sys.path:
        sys.path.insert(0, p)

from contextlib import ExitStack

import numpy as np
import ml_dtypes

import concourse.bass as bass
import concourse.mybir as mybir
import concourse.tile as tile
from concourse import bacc, bass_utils
from concourse.masks import make_identity

P = 128
N = 8192
NCORES = 8
NS = N // NCORES          # rows per core (1024)
JT = N // P               # j tiles (64)
CH = 8                    # j tiles per m DMA chunk
NCH = JT // CH            # m DMA chunks (8)
NPAIR = JT // 2           # DoubleRow pairs (32)
F = 128                   # hidden/emb width
C = 16                    # classes
VOCAB = 32768
G_LOCAL = NS // P         # graphs per core (8); graph == one 128-row block
GAIN = 64.0               # power-of-2 msg gain (64 = sqrt(nominal degree))
GE = 32.0                 # residual gain for the msg2 error stream

F32 = mybir.dt.float32
BF16 = mybir.dt.bfloat16
FP8 = mybir.dt.float8e4
I32 = mybir.dt.int32

NP_FP8 = ml_dtypes.float8_e4m3
NP_BF16 = ml_dtypes.bfloat16

_CACHE = {}

# When False, collectives are replaced by local DMA copies so the module can
# run under the single-core TimelineSim for bottleneck analysis. The graded
# path always builds with collectives.
_USE_COLLECTIVES = True

# Ship only the quantized msg2 stream (no fp8 residual): halves the msg2
# AllGather (the HW collective floor dominates the marginal rep time) and
# phase E. End-to-end rel err ~1.2e-2 vs the 2e-2 gate (numpy model matched
# HW within 2e-5 on previous configs).
_USE_RESIDUAL = False
S2 = 2 if _USE_RESIDUAL else 1

# cd-partial engine split per chunk (ACT, DVE) — must sum to CH.
_CD_SPLIT = {0: (4, 4), 1: (4, 4), 2: (4, 4), 3: (4, 4)}
_CD_SPLIT_DEFAULT = (3, 5)

# When False, the late closures are emitted inside their own rep (no
# software pipelining) — correctness bisect switch.
_DEFER = True

DR = mybir.MatmulPerfMode.DoubleRow
RELU = mybir.ActivationFunctionType.Relu
COPY = mybir.ActivationFunctionType.Copy
SQRT = mybir.ActivationFunctionType.Sqrt


def _build(reps=1):
    nc = bacc.Bacc("TRN2", target_bir_lowering=False, debug=False,
                   enable_asserts=True, num_devices=NCORES)

    mT_pre = nc.dram_tensor("mT_pre", [P, JT, NS], FP8, kind="ExternalInput")
    x_loc = nc.dram_tensor("x_loc", [P, G_LOCAL], I32, kind="ExternalInput")
    emb_in = nc.dram_tensor("emb_in", [VOCAB, F], BF16, kind="ExternalInput")
    w1_in = nc.dram_tensor("w1_in", [F, F], F32, kind="ExternalInput")
    b1_in = nc.dram_tensor("b1_in", [F], F32, kind="ExternalInput")
    w2_in = nc.dram_tensor("w2_in", [F, F], F32, kind="ExternalInput")
    b2_in = nc.dram_tensor("b2_in", [F], F32, kind="ExternalInput")
    wc_in = nc.dram_tensor("wc_in", [C, F], F32, kind="ExternalInput")
    bc_in = nc.dram_tensor("bc_in", [C], F32, kind="ExternalInput")
    out_l = nc.dram_tensor("out_l", [G_LOCAL, C], F32, kind="ExternalOutput")

    with tile.TileContext(nc) as tc, ExitStack() as stack:
        consts = stack.enter_context(tc.tile_pool(name="consts", bufs=1))
        big = stack.enter_context(tc.tile_pool(name="big", bufs=1))
        rep_sb = stack.enter_context(tc.tile_pool(name="rep_sb", bufs=2))
        dram = stack.enter_context(tc.tile_pool(name="dram", bufs=2, space="DRAM"))
        psum = stack.enter_context(tc.tile_pool(name="psum", bufs=1, space="PSUM"))
        ab_psum = stack.enter_context(
            tc.tile_pool(name="ab_psum", bufs=2, space="PSUM"))

        ident_bf = consts.tile([P, P], BF16)
        make_identity(nc, ident_bf)

        # ---- small constants -------------------------------------------
        ones_row = consts.tile([1, P], BF16)      # bias outer-product lhsT
        nc.vector.memset(ones_row[:], 1.0)
        ones8_f32 = consts.tile([1, G_LOCAL], F32)
        nc.vector.memset(ones8_f32[:], 1.0)
        # rd DoubleRow stationary: 16 ones-columns per K-plane (dual-fp8
        # LdWeights needs a 16B-aligned plane stride, so M=1 is illegal; the
        # 16 duplicate output rows cost no extra cycles)
        ones_pair = consts.tile([P, 2, 16], FP8)
        nc.vector.memset(ones_pair[:], 1.0)
        zs128 = consts.tile([P, P], FP8)
        nc.vector.memset(zs128[:], 0.0)
        zs16 = consts.tile([P, 16], FP8)
        nc.vector.memset(zs16[:], 0.0)
        b1_row = consts.tile([1, F], BF16)
        nc.gpsimd.dma_start(b1_row[:], b1_in.ap()[None, :])
        b2_row = consts.tile([1, F], BF16)
        nc.gpsimd.dma_start(b2_row[:], b2_in.ap()[None, :])
        bc_row = consts.tile([1, C], F32)
        nc.sync.dma_start(bc_row[:], bc_in.ap()[None, :])
        x_sb = consts.tile([P, G_LOCAL], I32)
        nc.sync.dma_start(x_sb[:], x_loc.ap())

        # w1T/w2T (transposed weights, bf16), wcT (f32)
        ident_f32 = consts.tile([C, C], F32)
        make_identity(nc, ident_f32)
        w1T = consts.tile([P, F], BF16)
        w2T = consts.tile([P, F], BF16)
        wcT = consts.tile([P, C], F32)
        with tc.tile_pool(name="wtmp", bufs=1) as wtmp:
            for w_in, wT in ((w1_in, w1T), (w2_in, w2T)):
                wf = wtmp.tile([F, F], F32, tag="wf")
                nc.sync.dma_start(wf[:], w_in.ap())
                wb = wtmp.tile([F, F], BF16, tag="wb")
                nc.vector.tensor_copy(wb[:], wf[:])
                ps = ab_psum.tile([P, F], BF16, tag="ab", name="wps")
                nc.tensor.transpose(ps[:], wb[:], ident_bf[:])
                nc.vector.tensor_copy(wT[:], ps[:])
            wcf = wtmp.tile([C, F], F32, tag="wcf")
            nc.sync.dma_start(wcf[:], wc_in.ap())
            pc = ab_psum.tile([P, C], F32, tag="ab", name="wcps")
            nc.tensor.transpose(pc[:], wcf[:], ident_f32[:])
            nc.vector.tensor_copy(wcT[:], pc[:])

        cn = dict(ident_bf=ident_bf, ones_row=ones_row, ones8_f32=ones8_f32,
                  ones_pair=ones_pair, zs128=zs128, zs16=zs16, b1_row=b1_row,
                  b2_row=b2_row, bc_row=bc_row, x_sb=x_sb, w1T=w1T, w2T=w2T,
                  wcT=wcT)
        late = {"sp_out": [], "tail": None}
        for _rep in range(reps):
            late = _emit_pipeline(
                nc, tc, big, rep_sb, dram, psum, ab_psum,
                mT_pre, emb_in, out_l, cn, late)
        # epilogue: flush the last rep's tail (AG2+readback, E, F, out)
        if late["tail"]:
            for f in late["tail"]:
                f()
        for f in late["sp_out"]:
            f()

    nc.compile()
    return nc


def _emit_pipeline(nc, tc, big, rep_sb, dram, psum, ab_psum,
                   mT_pre, emb_in, out_l, cn, late):
    ident_bf = cn["ident_bf"]
    nl = {"sp_out": list(late["sp_out"]), "tail": None}
    lt = late["tail"] or [None] * 8
    (lt_drelu, lt_ph, lt_e, lt_f1, lt_f2, lt_f3, lt_cls, lt_f5) = lt

    # ---- resident tensors ------------------------------------------
    mT = big.tile([P, JT, NS], FP8, tag="mT", name="mT", bufs=2)
    msg1s = big.tile([P, JT, F], FP8, tag="msg1s", name="msg1s", bufs=1)
    cd_acc = rep_sb.tile([P, JT], F32, tag="cd_acc", name="cd_acc")
    cd_scr = rep_sb.tile([P, NS], FP8, tag="cd_scr", name="cd_scr", bufs=1)

    # PSUM tags (8 banks): acc1 [t1T,t2b] | acc2 [zps_all,t2a] | rdbc [rd_ps]
    #                      | ab (2x rotating small)
    t1T = psum.tile([P, NS], F32, tag="acc1", name="t1T")
    rd_ps = psum.tile([16, NS], F32, tag="rdbc", name="rd_ps")

    # ---- SP: deferred out DMA (rep r-2), then this rep's m chunks --
    if len(nl["sp_out"]) >= 2:
        nl["sp_out"].pop(0)()
    for jc in range(NCH):
        sl = slice(jc * CH, (jc + 1) * CH)
        nc.sync.dma_start(mT[:, sl, :], mT_pre.ap()[:, sl, :])

    # ---- Pool: gathers first, then rep r-1's msg2 AG + readback ----
    h_bs = []
    for t in range(G_LOCAL):
        h_b = rep_sb.tile([P, F], BF16, tag=f"hb{t % 2}", name="h_b")
        nc.gpsimd.indirect_dma_start(
            out=h_b[:], out_offset=None, in_=emb_in[:],
            in_offset=bass.IndirectOffsetOnAxis(ap=cn["x_sb"][:, t:t + 1],
                                                axis=0),
        )
        h_bs.append(h_b)
    if lt_drelu:
        lt_drelu()
    if lt_ph:
        lt_ph()

    # ---- ACT: chunk-0 cd partials first (cd gates the RS->AG1->C
    # chain; chunk 0 lands before the gathers finish) -----------------
    na0, _ = _CD_SPLIT.get(0, _CD_SPLIT_DEFAULT)
    for k in range(na0):
        nc.scalar.activation(cd_scr[:, :], mT[:, k, :], COPY,
                             accum_out=cd_acc[:, k:k + 1])

    # ---- PE + ACT: phase B (transpose, GEMM, relu -> bf16 local) ---
    msg1b_loc = rep_sb.tile([P, G_LOCAL, F], BF16, tag="m1b", name="msg1b_loc")
    for t in range(G_LOCAL):
        tps = ab_psum.tile([P, P], BF16, tag="ab", name="tps")
        nc.tensor.transpose(tps[:], h_bs[t][:], ident_bf[:])
        hT = rep_sb.tile([P, F], BF16, tag=f"hT{t % 2}", name="hT")
        nc.scalar.copy(hT[:], tps[:])
        bps = ab_psum.tile([P, F], F32, tag="ab", name="bps")
        nc.tensor.matmul(bps[:], hT[:], cn["w1T"][:], start=True, stop=False)
        nc.tensor.matmul(bps[:], cn["ones_row"][:], cn["b1_row"][:],
                         start=False, stop=True)
        nc.scalar.activation(msg1b_loc[:, t, :], bps[:], RELU)

    # ---- PE: rd (zero pass + DoubleRow ones accumulation) ----------
    def _psum_zero(acc, zs):
        for k in range(CH):
            nc.tensor.matmul(acc[:, k * P:(k + 1) * P], zs, ident_bf[:],
                             start=True, stop=False, skip_group_check=True)

    _psum_zero(rd_ps, cn["zs16"])
    for q in range(NPAIR):
        j0 = 2 * q
        for h in range(4):
            hs = slice(h * 256, (h + 1) * 256)
            nc.tensor.matmul(
                rd_ps[:, hs], cn["ones_pair"][:], mT[:, j0:j0 + 2, hs],
                start=False, stop=(q == NPAIR - 1), perf_mode=DR,
                skip_group_check=True)

    # rep r-1's phase E rides the PE queue here (its msg2 readback was
    # emitted above on the Pool queue)
    if lt_e:
        lt_e()

    # ---- ACT/DVE: cd partials, chunk-paced -------------------------
    for jc in range(NCH):
        na, nd = _CD_SPLIT.get(jc, _CD_SPLIT_DEFAULT)
        base = jc * CH
        for k in range(0 if jc else na, na):
            jt = base + k
            nc.scalar.activation(cd_scr[:, :], mT[:, jt, :], COPY,
                                 accum_out=cd_acc[:, jt:jt + 1])
        if nd:
            nc.vector.reduce_sum(out=cd_acc[:, base + na:base + CH],
                                 in_=mT[:, base + na:base + CH, :],
                                 axis=mybir.AxisListType.X)

    # ---- rd/cd scale rows (packed on partitions 0-2 of one tile;
    # beta/alpha live in their own partition-0 tiles for the broadcasts) ----
    srow_t = rep_sb.tile([1, NS], F32, tag="srow", name="srow_t", bufs=1)
    cdr_t = rep_sb.tile([1, NS], F32, tag="cdr", name="cdr_t", bufs=1)
    s64_t = rep_sb.tile([1, NS], F32, tag="s64", name="s64_t", bufs=1)
    srow = srow_t[:]
    cdr_row = cdr_t[:]
    scl64_row = s64_t[:]
    nc.scalar.activation(srow, rd_ps[0:1, :], SQRT, scale=GAIN * GAIN)
    beta_row = rep_sb.tile([1, NS], F32, tag="beta", name="beta_row", bufs=1)
    nc.vector.reciprocal(beta_row[:], srow)

    cd_part = dram.tile([N], F32, tag="cd_part", name="cd_part")
    cd_loc = dram.tile([NS], F32, tag="cd_loc", name="cd_loc")
    nc.sync.dma_start(cd_part[:].rearrange("(t p) -> p t", p=P), cd_acc[:])
    if _USE_COLLECTIVES:
        nc.gpsimd.collective_compute(
            "ReduceScatter", mybir.AluOpType.add,
            replica_groups=[list(range(NCORES))],
            ins=[cd_part.opt()], outs=[cd_loc.opt()],
        )
    else:
        nc.gpsimd.dma_start(cd_loc[:], cd_part[0:NS])

    cdl_sb = rep_sb.tile([P, G_LOCAL], F32, tag="cdl", name="cdl_sb")
    nc.sync.dma_start(cdl_sb[:], cd_loc[:].rearrange("(t p) -> p t", p=P))
    nc.sync.dma_start(cdr_row, cd_loc[:][None, :])

    # scl_t = G*s_cl tiles; scl64/alpha rows; bf16 bias row
    scl_t = rep_sb.tile([P, G_LOCAL], F32, tag="scl", name="scl_t")
    nc.scalar.activation(scl_t[:], cdl_sb[:], SQRT, scale=1.0 / (GAIN * GAIN))
    nc.vector.reciprocal(scl_t[:], scl_t[:])
    nc.scalar.activation(scl64_row, cdr_row, SQRT,
                         scale=1.0 / (GAIN * GAIN))
    nc.vector.reciprocal(scl64_row, scl64_row)
    scl64_bf = rep_sb.tile([1, NS], BF16, tag="s64b", name="scl64_bf", bufs=1)
    nc.scalar.copy(scl64_bf[:], scl64_row)
    alpha_row = rep_sb.tile([1, NS], F32, tag="arow", name="alpha_row", bufs=1)
    nc.vector.tensor_mul(alpha_row[:], scl64_row, beta_row[:])

    # ---- ACT: scale local msg1 -> fp8, ship, AllGather -------------
    msg1q_loc = rep_sb.tile([P, G_LOCAL, F], FP8, tag="m1q", name="msg1q_loc")
    for t in range(G_LOCAL):
        nc.vector.tensor_scalar_mul(msg1q_loc[:, t, :], msg1b_loc[:, t, :],
                                    scl_t[:, t:t + 1])
    msg1_loc_d = dram.tile([P, G_LOCAL, F], FP8, tag="m1ld", name="msg1_loc_d")
    msg1_full = dram.tile([NCORES * P, G_LOCAL, F], FP8, tag="m1f",
                          name="msg1_full", addr_space="Shared")
    nc.gpsimd.dma_start(msg1_loc_d[:], msg1q_loc[:])
    if _USE_COLLECTIVES:
        nc.gpsimd.collective_compute(
            "AllGather", mybir.AluOpType.bypass,
            replica_groups=[list(range(NCORES))],
            ins=[msg1_loc_d.opt()], outs=[msg1_full.opt()],
        )
    else:
        scr1 = dram.tile([NCORES * P, G_LOCAL, F], FP8, tag="m1scr",
                         name="m1scr")
        nc.gpsimd.dma_start(scr1[0:P, :, :], msg1_loc_d[:])
        nc.gpsimd.dma_start(msg1_full[:], scr1[:])

    # ---- rep r-1 phase F (dataflow order: dve comb -> pool mul ->
    # dve segmax -> pe classifier -> dve out copy), interleaved so the
    # bcR broadcast never WAR-waits on a later op in its own Pool queue ----
    if lt_f1:
        lt_f1()
    if lt_f2:
        lt_f2()
    alpha_bc = rep_sb.tile([P, NS], F32, tag="abc", name="alpha_bc", bufs=1)
    nc.gpsimd.partition_broadcast(alpha_bc[:], alpha_row[:])
    bcR_sb = rep_sb.tile([P, NS], F32, tag="bcR", name="bcR_sb", bufs=1)
    nc.gpsimd.partition_broadcast(bcR_sb[:], beta_row[:])
    if lt_f3:
        lt_f3()
    if lt_cls:
        lt_cls()
    if lt_f5:
        lt_f5()

    # ---- ACT: msg1s readback (scalar HWDGE) ------------------------
    nc.sync.dma_start(
        msg1s[:].rearrange("p (kc t) f -> p kc t f", kc=NCORES),
        msg1_full[:].rearrange("(kc p) t f -> p kc t f", p=P))

    # ---- PE: phase C ----------------------------------------------
    _psum_zero(t1T, cn["zs128"])
    for q in range(NPAIR):
        j0 = 2 * q
        for h in range(4):
            hs = slice(h * 256, (h + 1) * 256)
            nc.tensor.matmul(
                t1T[:, hs], msg1s[:, j0:j0 + 2, :], mT[:, j0:j0 + 2, hs],
                start=False, stop=(q == NPAIR - 1), perf_mode=DR,
                skip_group_check=True)

    # ---- phase D: batched msg2 (q + residual) ----------------------
    t1sbs = rep_sb.tile([P, NS], BF16, tag="t1sbs", name="t1sbs", bufs=1)
    nc.vector.tensor_mul(t1sbs[:], t1T[:], alpha_bc[:])

    zps_all = psum.tile([P, NS], F32, tag="acc2", name="zps_all")
    _psum_zero(zps_all, cn["zs128"])
    for t in range(G_LOCAL):
        ts = slice(t * P, (t + 1) * P)
        nc.tensor.matmul(zps_all[:, ts], t1sbs[:, ts], cn["w2T"][:],
                         start=False, stop=False, skip_group_check=True)
        nc.tensor.matmul(zps_all[:, ts], scl64_bf[:, ts], cn["b2_row"][:],
                         start=False, stop=True, skip_group_check=True)

    msg2p = rep_sb.tile([P, G_LOCAL, S2, F], FP8, tag="m2p", name="msg2p",
                        bufs=1)
    zview = zps_all[:].rearrange("p (t f) -> p t f", t=G_LOCAL)

    def _t_drelu():
        nc.scalar.activation(msg2p[:, :, 0, :], zview, RELU)

    if not _DEFER:
        _t_drelu()
    if _USE_RESIDUAL:
        # residual err = GE*(relu(z) - q): both operands pre-scaled by GE so
        # the subtract writes the fp8 err stream directly
        m2ball = rep_sb.tile([P, NS], BF16, tag="m1b", name="m2ball", bufs=2)
        nc.scalar.activation(m2ball[:], zps_all[:], RELU, scale=GE)
        m2dall = rep_sb.tile([P, NS], BF16, tag="t1sbs", name="m2dall",
                             bufs=1)
        nc.vector.tensor_scalar_mul(
            m2dall[:].rearrange("p (t f) -> p t f", t=G_LOCAL),
            msg2p[:, :, 0, :], GE)
        nc.vector.tensor_sub(
            msg2p[:, :, 1, :],
            m2ball[:].rearrange("p (t f) -> p t f", t=G_LOCAL),
            m2dall[:].rearrange("p (t f) -> p t f", t=G_LOCAL))

    # ---- rep r's tail: msg2 AG + readback, phase E, phase F --------
    # All emitted inside rep r+1's streams (or the epilogue) so every read
    # follows its producer in emission order.
    msg2_loc_d = dram.tile([P, G_LOCAL, S2, F], FP8, tag="m2ld",
                           name="msg2_loc_d")
    msg2_full = dram.tile([NCORES * P, G_LOCAL, S2, F], FP8, tag="m2f",
                          name="msg2_full", addr_space="Shared")
    m2full = rep_sb.tile([P, JT, S2, F], FP8, tag="m2full", name="m2full",
                         bufs=1)
    if _USE_RESIDUAL:
        t2s = rep_sb.tile([P, NS], F32, tag="t2hs", name="t2s", bufs=2)
        h2a = rep_sb.tile([P, NS], F32, tag="t2hs", name="h2a", bufs=2)
    h2s = rep_sb.tile([P, NS], F32, tag="t2hs", name="h2s", bufs=2)
    pooledT = rep_sb.tile([P, G_LOCAL], F32, tag="pooledT", name="pooledT")
    out_sb = rep_sb.tile([G_LOCAL, C], F32, tag="out_sb", name="out_sb",
                         bufs=3)
    box = {}

    def _t_pool_head():
        nc.gpsimd.dma_start(msg2_loc_d[:], msg2p[:])
        if _USE_COLLECTIVES:
            nc.gpsimd.collective_compute(
                "AllGather", mybir.AluOpType.bypass,
                replica_groups=[list(range(NCORES))],
                ins=[msg2_loc_d.opt()], outs=[msg2_full.opt()],
            )
        else:
            scr2 = dram.tile([NCORES * P, G_LOCAL, S2, F], FP8, tag="m2scr",
                             name="m2scr")
            nc.gpsimd.dma_start(scr2[0:P, :, :, :], msg2_loc_d[:])
            nc.gpsimd.dma_start(msg2_full[:], scr2[:])
        nc.gpsimd.dma_start(
            m2full[:].rearrange("p (kc t) s f -> p kc t s f", kc=NCORES),
            msg2_full[:].rearrange("(kc p) t s f -> p kc t s f", p=P))

    def _t_e():
        t2a = psum.tile([P, NS], F32, tag="acc2", name="t2a")
        box["t2a"] = t2a
        _psum_zero(t2a, cn["zs128"])
        if _USE_RESIDUAL:
            t2b = psum.tile([P, NS], F32, tag="acc1", name="t2b")
            box["t2b"] = t2b
            _psum_zero(t2b, cn["zs128"])
        for q in range(NPAIR):
            j0 = 2 * q
            for h in range(4):
                hs = slice(h * 256, (h + 1) * 256)
                nc.tensor.matmul(
                    t2a[:, hs], m2full[:, j0:j0 + 2, 0, :],
                    mT[:, j0:j0 + 2, hs],
                    start=False, stop=(q == NPAIR - 1), perf_mode=DR,
                    skip_group_check=True)
            if _USE_RESIDUAL:
                for h in range(4):
                    hs = slice(h * 256, (h + 1) * 256)
                    nc.tensor.matmul(
                        box["t2b"][:, hs], m2full[:, j0:j0 + 2, 1, :],
                        mT[:, j0:j0 + 2, hs],
                        start=False, stop=(q == NPAIR - 1), perf_mode=DR,
                        skip_group_check=True)

    def _t_f1():
        if _USE_RESIDUAL:
            nc.vector.tensor_scalar_mul(t2s[:], box["t2b"][:], 1.0 / GE)
            nc.vector.tensor_add(h2a[:], t2s[:], box["t2a"][:])
        else:
            # single stream: h2s = t2a * bcR directly on DVE (one PSUM input)
            nc.vector.tensor_mul(h2s[:], box["t2a"][:], bcR_sb[:])

    def _t_f2():
        if _USE_RESIDUAL:
            nc.gpsimd.tensor_mul(h2s[:], h2a[:], bcR_sb[:])

    def _t_f3():
        for g in range(G_LOCAL):
            nc.vector.reduce_max(out=pooledT[:, g:g + 1],
                                 in_=h2s[:, g * P:(g + 1) * P],
                                 axis=mybir.AxisListType.X)

    def _t_cls():
        cps = ab_psum.tile([G_LOCAL, C], F32, tag="ab", name="cps")
        box["cps"] = cps
        nc.tensor.matmul(cps[:], pooledT[:], cn["wcT"][:],
                         start=True, stop=False)
        nc.tensor.matmul(cps[:], cn["ones8_f32"][:], cn["bc_row"][:],
                         start=False, stop=True)

    def _t_f5():
        nc.vector.tensor_copy(out_sb[:], box["cps"][:])

    def _sp_out():
        nc.sync.dma_start(out_l.ap(), out_sb[:])

    if _DEFER:
        nl["tail"] = [_t_drelu, _t_pool_head, _t_e, _t_f1, _t_f2, _t_f3,
                      _t_cls, _t_f5]
        nl["sp_out"].append(_sp_out)
    else:
        for f in (_t_pool_head, _t_e, _t_f1, _t_f2, _t_f3, _t_cls, _t_f5,
                  _sp_out):
            f()
    return nl


def _get_nc():
    if "nc" not in _CACHE:
        _CACHE["nc"] = _build()
    return _CACHE["nc"]


def _prep_in_maps(inputs):
    m = np.asarray(inputs["m"], dtype=np.float32)
    x = np.asarray(inputs["x"]).astype(np.int32)
    emb = np.asarray(inputs["emb"], dtype=np.float32).astype(NP_BF16)
    w1 = np.ascontiguousarray(np.asarray(inputs["w1"], dtype=np.float32))
    b1 = np.ascontiguousarray(np.asarray(inputs["b1"], dtype=np.float32))
    w2 = np.ascontiguousarray(np.asarray(inputs["w2"], dtype=np.float32))
    b2 = np.ascontiguousarray(np.asarray(inputs["b2"], dtype=np.float32))
    wc = np.ascontiguousarray(np.asarray(inputs["wc"], dtype=np.float32))
    bc = np.ascontiguousarray(np.asarray(inputs["bc"], dtype=np.float32))

    in_maps = []
    for k in range(NCORES):
        # mT_pre[p, jt, i] = m[k*NS + i, jt*P + p], cast to fp8e4m3
        shard = m[k * NS:(k + 1) * NS, :]                      # [i, j]
        mt = np.ascontiguousarray(
            shard.T.reshape(JT, P, NS).transpose(1, 0, 2)).astype(NP_FP8)
        # x_loc[p, t] = x[k*NS + t*128 + p]
        xl = np.ascontiguousarray(
            x[k * NS:(k + 1) * NS].reshape(G_LOCAL, P).T)
        in_maps.append({
            "mT_pre": mt, "x_loc": xl, "emb_in": emb,
            "w1_in": w1, "b1_in": b1, "w2_in": w2, "b2_in": b2,
            "wc_in": wc, "bc_in": bc,
        })
    return in_maps


def kernel(**inputs):
    nc = _get_nc()
    in_maps = _prep_in_maps(inputs)
    res = bass_utils.run_bass_kernel_spmd(
        nc, in_maps, core_ids=list(range(NCORES)))
    out = np.concatenate([res.results[k]["out_l"] for k in range(NCORES)], axis=0)
    return out.astype(np.float32)


# revision 67
# speedup vs baseline: 1.0215x; 1.0215x over previous
"""GCN message-passing kernel for Trainium2, 8-core SPMD — v4 (fp8, software-
pipelined emission).

Data plan (per core, NS=1024 local rows, JT=64 node tiles, fp8 m):
  * m pre-cast to fp8e4m3 on the HOST (layout/dtype prep only): 8 MB/core per
    rep (~24 us window at ~350 GB/s).
  * Both SpMM layers are fp8 DoubleRow matmuls (two j-tiles contracted per
    pass). Messages carry a power-of-2 gain G=64 folded exactly into the
    degree scales so fp8 stays in the normal range. An optional fp8 residual
    stream for msg2 (_USE_RESIDUAL) buys ~1e-3 rel err at +1 MB of AllGather
    and 2x phase E; it is OFF by default (rel err ~1.2e-2 vs the 2e-2 gate)
    because the serialized HW collective latency dominates the marginal rep
    time (~51 us/rep measured for RS32K+AG1M+AG2M).
  * PSUM accumulators are zeroed by explicit zero-stationary matmul passes;
    all accumulating matmuls use start=False (a start=True write zeroes the
    whole 2KB bank region, which would wipe interleaved 1KB slices).
  * msg1 computed for local nodes only, scaled by G*s_cl (one ReduceScatter
    of the column-degree partials), AllGathered as fp8.
  * Phase D folds alpha = s_cl*s_r into the t1 stationary via a
    partition-broadcast of the alpha row, so relu+quantize run as two batched
    ACT ops over [128,1024] (no per-tile scales, no activation-table thrash).
  * Column-degree partials split ACT (Copy+accum_out) / DVE (reduce) per
    chunk; rd via DoubleRow ones-matmuls on PE.

Scheduling plan: engines execute their instruction streams in order, so late
ops of rep r (msg2 AllGather + readback, phase F elementwise, classifier, out
DMA) are EMITTED inside rep r+1/r+2's streams. This keeps every queue's
emission order consistent with execution order across rep boundaries and lets
rep r+1's m-window overlap rep r's tail (the marginal rep time is what the
harness measures).
  SP/sync : [out dma (r-2)] [m chunks] [cd_part out] [cdl tiles/row in]
            [msg1s readback]
  Pool    : [gathers] [msg2 out+AG2+readback (r-1)] [RS] [msg1q out] [AG1]
            [alpha/beta broadcasts]
  ACT     : [B relus + hT copies] [cd share] [sqrts] [scale8] [D relus]
  DVE     : [cd share] [recips/alpha] [F mul + segmax + out copy (r-1)]
            [t1*alpha]
  PE      : [B transp/GEMM] [rd] [E (r-1)] [classifier (r-1)] [C] [D GEMMs]
"""

import sys

for p in ("/opt/trn_rl_repo",):
    if p not in sys.path:
        sys.path.insert(0, p)

from contextlib import ExitStack

import numpy as np
import ml_dtypes

import concourse.bass as bass
import concourse.mybir as mybir
import concourse.tile as tile
from concourse import bacc, bass_utils
from concourse.masks import make_identity

P = 128
N = 8192
NCORES = 8
NS = N // NCORES          # rows per core (1024)
JT = N // P               # j tiles (64)
CH = 8                    # j tiles per m DMA chunk
NCH = JT // CH            # m DMA chunks (8)
NPAIR = JT // 2           # DoubleRow pairs (32)
F = 128                   # hidden/emb width
C = 16                    # classes
VOCAB = 32768
G_LOCAL = NS // P         # graphs per core (8); graph == one 128-row block
GAIN = 64.0               # power-of-2 msg gain (64 = sqrt(nominal degree))
GE = 32.0                 # residual gain for the msg2 error stream

F32 = mybir.dt.float32
BF16 = mybir.dt.bfloat16
FP8 = mybir.dt.float8e4
I32 = mybir.dt.int32

NP_FP8 = ml_dtypes.float8_e4m3
NP_BF16 = ml_dtypes.bfloat16

_CACHE = {}

# When False, collectives are replaced by local DMA copies so the module can
# run under the single-core TimelineSim for bottleneck analysis. The graded
# path always builds with collectives.
_USE_COLLECTIVES = True

# Ship only the quantized msg2 stream (no fp8 residual): halves the msg2
# AllGather (the HW collective floor dominates the marginal rep time) and
# phase E. End-to-end rel err ~1.2e-2 vs the 2e-2 gate (numpy model matched
# HW within 2e-5 on previous configs).
_USE_RESIDUAL = False
S2 = 2 if _USE_RESIDUAL else 1

# cd-partial engine split per chunk (ACT, DVE) — must sum to CH.
_CD_SPLIT = {0: (4, 4), 1: (4, 4), 2: (4, 4), 3: (4, 4)}
_CD_SPLIT_DEFAULT = (3, 5)

# When False, the late closures are emitted inside their own rep (no
# software pipelining) — correctness bisect switch.
_DEFER = True

DR = mybir.MatmulPerfMode.DoubleRow
RELU = mybir.ActivationFunctionType.Relu
COPY = mybir.ActivationFunctionType.Copy
SQRT = mybir.ActivationFunctionType.Sqrt


def _build(reps=1):
    nc = bacc.Bacc("TRN2", target_bir_lowering=False, debug=False,
                   enable_asserts=True, num_devices=NCORES)

    mT_pre = nc.dram_tensor("mT_pre", [P, JT, NS], FP8, kind="ExternalInput")
    x_loc = nc.dram_tensor("x_loc", [P, G_LOCAL], I32, kind="ExternalInput")
    emb_in = nc.dram_tensor("emb_in", [VOCAB, F], BF16, kind="ExternalInput")
    w1_in = nc.dram_tensor("w1_in", [F, F], F32, kind="ExternalInput")
    b1_in = nc.dram_tensor("b1_in", [F], F32, kind="ExternalInput")
    w2_in = nc.dram_tensor("w2_in", [F, F], F32, kind="ExternalInput")
    b2_in = nc.dram_tensor("b2_in", [F], F32, kind="ExternalInput")
    wc_in = nc.dram_tensor("wc_in", [C, F], F32, kind="ExternalInput")
    bc_in = nc.dram_tensor("bc_in", [C], F32, kind="ExternalInput")
    out_l = nc.dram_tensor("out_l", [G_LOCAL, C], F32, kind="ExternalOutput")

    with tile.TileContext(nc) as tc, ExitStack() as stack:
        consts = stack.enter_context(tc.tile_pool(name="consts", bufs=1))
        big = stack.enter_context(tc.tile_pool(name="big", bufs=1))
        rep_sb = stack.enter_context(tc.tile_pool(name="rep_sb", bufs=2))
        dram = stack.enter_context(tc.tile_pool(name="dram", bufs=2, space="DRAM"))
        psum = stack.enter_context(tc.tile_pool(name="psum", bufs=1, space="PSUM"))
        ab_psum = stack.enter_context(
            tc.tile_pool(name="ab_psum", bufs=2, space="PSUM"))

        ident_bf = consts.tile([P, P], BF16)
        make_identity(nc, ident_bf)

        # ---- small constants -------------------------------------------
        ones_row = consts.tile([1, P], BF16)      # bias outer-product lhsT
        nc.vector.memset(ones_row[:], 1.0)
        ones8_f32 = consts.tile([1, G_LOCAL], F32)
        nc.vector.memset(ones8_f32[:], 1.0)
        # rd DoubleRow stationary: 16 ones-columns per K-plane (dual-fp8
        # LdWeights needs a 16B-aligned plane stride, so M=1 is illegal; the
        # 16 duplicate output rows cost no extra cycles)
        ones_pair = consts.tile([P, 2, 16], FP8)
        nc.vector.memset(ones_pair[:], 1.0)
        zs128 = consts.tile([P, P], FP8)
        nc.vector.memset(zs128[:], 0.0)
        zs16 = consts.tile([P, 16], FP8)
        nc.vector.memset(zs16[:], 0.0)
        b1_row = consts.tile([1, F], BF16)
        nc.gpsimd.dma_start(b1_row[:], b1_in.ap()[None, :])
        b2_row = consts.tile([1, F], BF16)
        nc.gpsimd.dma_start(b2_row[:], b2_in.ap()[None, :])
        bc_row = consts.tile([1, C], F32)
        nc.sync.dma_start(bc_row[:], bc_in.ap()[None, :])
        x_sb = consts.tile([P, G_LOCAL], I32)
        nc.sync.dma_start(x_sb[:], x_loc.ap())

        # w1T/w2T (transposed weights, bf16), wcT (f32)
        ident_f32 = consts.tile([C, C], F32)
        make_identity(nc, ident_f32)
        w1T = consts.tile([P, F], BF16)
        w2T = consts.tile([P, F], BF16)
        wcT = consts.tile([P, C], F32)
        with tc.tile_pool(name="wtmp", bufs=1) as wtmp:
            for w_in, wT in ((w1_in, w1T), (w2_in, w2T)):
                wf = wtmp.tile([F, F], F32, tag="wf")
                nc.sync.dma_start(wf[:], w_in.ap())
                wb = wtmp.tile([F, F], BF16, tag="wb")
                nc.vector.tensor_copy(wb[:], wf[:])
                ps = ab_psum.tile([P, F], BF16, tag="ab", name="wps")
                nc.tensor.transpose(ps[:], wb[:], ident_bf[:])
                nc.vector.tensor_copy(wT[:], ps[:])
            wcf = wtmp.tile([C, F], F32, tag="wcf")
            nc.sync.dma_start(wcf[:], wc_in.ap())
            pc = ab_psum.tile([P, C], F32, tag="ab", name="wcps")
            nc.tensor.transpose(pc[:], wcf[:], ident_f32[:])
            nc.vector.tensor_copy(wcT[:], pc[:])

        cn = dict(ident_bf=ident_bf, ones_row=ones_row, ones8_f32=ones8_f32,
                  ones_pair=ones_pair, zs128=zs128, zs16=zs16, b1_row=b1_row,
                  b2_row=b2_row, bc_row=bc_row, x_sb=x_sb, w1T=w1T, w2T=w2T,
                  wcT=wcT)
        late = {"sp_out": [], "tail": None}
        for _rep in range(reps):
            late = _emit_pipeline(
                nc, tc, big, rep_sb, dram, psum, ab_psum,
                mT_pre, emb_in, out_l, cn, late)
        # epilogue: flush the last rep's tail (AG2+readback, E, F, out)
        if late["tail"]:
            for f in late["tail"]:
                f()
        for f in late["sp_out"]:
            f()

    nc.compile()
    return nc


def _emit_pipeline(nc, tc, big, rep_sb, dram, psum, ab_psum,
                   mT_pre, emb_in, out_l, cn, late):
    ident_bf = cn["ident_bf"]
    nl = {"sp_out": list(late["sp_out"]), "tail": None}
    lt = late["tail"] or [None] * 8
    (lt_drelu, lt_ph, lt_e, lt_f1, lt_f2, lt_f3, lt_cls, lt_f5) = lt

    # ---- resident tensors ------------------------------------------
    mT = big.tile([P, JT, NS], FP8, tag="mT", name="mT", bufs=2)
    msg1s = big.tile([P, JT, F], FP8, tag="msg1s", name="msg1s", bufs=1)
    cd_acc = rep_sb.tile([P, JT], F32, tag="cd_acc", name="cd_acc")
    cd_scr = rep_sb.tile([P, NS], FP8, tag="cd_scr", name="cd_scr", bufs=1)

    # PSUM tags (8 banks): acc1 [t1T,t2b] | acc2 [zps_all,t2a] | rdbc [rd_ps]
    #                      | ab (2x rotating small)
    t1T = psum.tile([P, NS], F32, tag="acc1", name="t1T")
    rd_ps = psum.tile([16, NS], F32, tag="rdbc", name="rd_ps")

    # ---- SP: deferred out DMA (rep r-2), then this rep's m chunks --
    if len(nl["sp_out"]) >= 2:
        nl["sp_out"].pop(0)()
    for jc in range(NCH):
        sl = slice(jc * CH, (jc + 1) * CH)
        nc.sync.dma_start(mT[:, sl, :], mT_pre.ap()[:, sl, :])

    # ---- Pool: gathers first, then rep r-1's msg2 AG + readback ----
    h_bs = []
    for t in range(G_LOCAL):
        h_b = rep_sb.tile([P, F], BF16, tag=f"hb{t % 2}", name="h_b")
        nc.gpsimd.indirect_dma_start(
            out=h_b[:], out_offset=None, in_=emb_in[:],
            in_offset=bass.IndirectOffsetOnAxis(ap=cn["x_sb"][:, t:t + 1],
                                                axis=0),
        )
        h_bs.append(h_b)
    if lt_drelu:
        lt_drelu()
    if lt_ph:
        lt_ph()

    # ---- ACT: chunk-0 cd partials first (cd gates the RS->AG1->C
    # chain; chunk 0 lands before the gathers finish) -----------------
    na0, _ = _CD_SPLIT.get(0, _CD_SPLIT_DEFAULT)
    for k in range(na0):
        nc.scalar.activation(cd_scr[:, :], mT[:, k, :], COPY,
                             accum_out=cd_acc[:, k:k + 1])

    # ---- PE + ACT: phase B (transpose, GEMM, relu -> bf16 local) ---
    msg1b_loc = rep_sb.tile([P, G_LOCAL, F], BF16, tag="m1b", name="msg1b_loc")
    for t in range(G_LOCAL):
        tps = ab_psum.tile([P, P], BF16, tag="ab", name="tps")
        nc.tensor.transpose(tps[:], h_bs[t][:], ident_bf[:])
        hT = rep_sb.tile([P, F], BF16, tag=f"hT{t % 2}", name="hT")
        nc.scalar.copy(hT[:], tps[:])
        bps = ab_psum.tile([P, F], F32, tag="ab", name="bps")
        nc.tensor.matmul(bps[:], hT[:], cn["w1T"][:], start=True, stop=False)
        nc.tensor.matmul(bps[:], cn["ones_row"][:], cn["b1_row"][:],
                         start=False, stop=True)
        nc.scalar.activation(msg1b_loc[:, t, :], bps[:], RELU)

    # ---- PE: rd (zero pass + DoubleRow ones accumulation) ----------
    def _psum_zero(acc, zs):
        for k in range(CH):
            nc.tensor.matmul(acc[:, k * P:(k + 1) * P], zs, ident_bf[:],
                             start=True, stop=False, skip_group_check=True)

    _psum_zero(rd_ps, cn["zs16"])
    for q in range(NPAIR):
        j0 = 2 * q
        for h in range(4):
            hs = slice(h * 256, (h + 1) * 256)
            nc.tensor.matmul(
                rd_ps[:, hs], cn["ones_pair"][:], mT[:, j0:j0 + 2, hs],
                start=False, stop=(q == NPAIR - 1), perf_mode=DR,
                skip_group_check=True)

    # rep r-1's phase E rides the PE queue here (its msg2 readback was
    # emitted above on the Pool queue)
    if lt_e:
        lt_e()

    # ---- ACT/DVE: cd partials, chunk-paced -------------------------
    for jc in range(NCH):
        na, nd = _CD_SPLIT.get(jc, _CD_SPLIT_DEFAULT)
        base = jc * CH
        for k in range(0 if jc else na, na):
            jt = base + k
            nc.scalar.activation(cd_scr[:, :], mT[:, jt, :], COPY,
                                 accum_out=cd_acc[:, jt:jt + 1])
        if nd:
            nc.vector.reduce_sum(out=cd_acc[:, base + na:base + CH],
                                 in_=mT[:, base + na:base + CH, :],
                                 axis=mybir.AxisListType.X)

    # ---- rd/cd scale rows (packed on partitions 0-2 of one tile;
    # beta/alpha live in their own partition-0 tiles for the broadcasts) ----
    srow_t = rep_sb.tile([1, NS], F32, tag="srow", name="srow_t", bufs=1)
    cdr_t = rep_sb.tile([1, NS], F32, tag="cdr", name="cdr_t", bufs=1)
    s64_t = rep_sb.tile([1, NS], F32, tag="s64", name="s64_t", bufs=1)
    srow = srow_t[:]
    cdr_row = cdr_t[:]
    scl64_row = s64_t[:]
    nc.scalar.activation(srow, rd_ps[0:1, :], SQRT, scale=GAIN * GAIN)
    beta_row = rep_sb.tile([1, NS], F32, tag="beta", name="beta_row", bufs=1)
    nc.vector.reciprocal(beta_row[:], srow)

    cd_part = dram.tile([N], F32, tag="cd_part", name="cd_part")
    cd_loc = dram.tile([NS], F32, tag="cd_loc", name="cd_loc")
    nc.sync.dma_start(cd_part[:].rearrange("(t p) -> p t", p=P), cd_acc[:])
    if _USE_COLLECTIVES:
        nc.gpsimd.collective_compute(
            "ReduceScatter", mybir.AluOpType.add,
            replica_groups=[list(range(NCORES))],
            ins=[cd_part.opt()], outs=[cd_loc.opt()],
        )
    else:
        nc.gpsimd.dma_start(cd_loc[:], cd_part[0:NS])

    cdl_sb = rep_sb.tile([P, G_LOCAL], F32, tag="cdl", name="cdl_sb")
    nc.sync.dma_start(cdl_sb[:], cd_loc[:].rearrange("(t p) -> p t", p=P))
    nc.sync.dma_start(cdr_row, cd_loc[:][None, :])

    # scl_t = G*s_cl tiles; scl64/alpha rows; bf16 bias row
    scl_t = rep_sb.tile([P, G_LOCAL], F32, tag="scl", name="scl_t")
    nc.scalar.activation(scl_t[:], cdl_sb[:], SQRT, scale=1.0 / (GAIN * GAIN))
    nc.vector.reciprocal(scl_t[:], scl_t[:])
    nc.scalar.activation(scl64_row, cdr_row, SQRT,
                         scale=1.0 / (GAIN * GAIN))
    nc.vector.reciprocal(scl64_row, scl64_row)
    scl64_bf = rep_sb.tile([1, NS], BF16, tag="s64b", name="scl64_bf", bufs=1)
    nc.scalar.copy(scl64_bf[:], scl64_row)
    alpha_row = rep_sb.tile([1, NS], F32, tag="arow", name="alpha_row", bufs=1)
    nc.vector.tensor_mul(alpha_row[:], scl64_row, beta_row[:])

    # ---- ACT: scale local msg1 -> fp8, ship, AllGather -------------
    msg1q_loc = rep_sb.tile([P, G_LOCAL, F], FP8, tag="m1q", name="msg1q_loc")
    for t in range(G_LOCAL):
        nc.scalar.activation(msg1q_loc[:, t, :], msg1b_loc[:, t, :], COPY,
                             scale=scl_t[:, t:t + 1])
    msg1_loc_d = dram.tile([P, G_LOCAL, F], FP8, tag="m1ld", name="msg1_loc_d")
    msg1_full = dram.tile([NCORES * P, G_LOCAL, F], FP8, tag="m1f",
                          name="msg1_full", addr_space="Shared")
    nc.gpsimd.dma_start(msg1_loc_d[:], msg1q_loc[:])
    if _USE_COLLECTIVES:
        nc.gpsimd.collective_compute(
            "AllGather", mybir.AluOpType.bypass,
            replica_groups=[list(range(NCORES))],
            ins=[msg1_loc_d.opt()], outs=[msg1_full.opt()],
        )
    else:
        scr1 = dram.tile([NCORES * P, G_LOCAL, F], FP8, tag="m1scr",
                         name="m1scr")
        nc.gpsimd.dma_start(scr1[0:P, :, :], msg1_loc_d[:])
        nc.gpsimd.dma_start(msg1_full[:], scr1[:])

    # ---- rep r-1 phase F (dataflow order: dve comb -> pool mul ->
    # dve segmax -> pe classifier -> dve out copy), interleaved so the
    # bcR broadcast never WAR-waits on a later op in its own Pool queue ----
    if lt_f1:
        lt_f1()
    if lt_f2:
        lt_f2()
    alpha_bc = rep_sb.tile([P, NS], F32, tag="abc", name="alpha_bc", bufs=1)
    nc.gpsimd.partition_broadcast(alpha_bc[:], alpha_row[:])
    bcR_sb = rep_sb.tile([P, NS], F32, tag="bcR", name="bcR_sb", bufs=1)
    nc.gpsimd.partition_broadcast(bcR_sb[:], beta_row[:])
    if lt_f3:
        lt_f3()
    if lt_cls:
        lt_cls()
    if lt_f5:
        lt_f5()

    # ---- ACT: msg1s readback (scalar HWDGE) ------------------------
    nc.sync.dma_start(
        msg1s[:].rearrange("p (kc t) f -> p kc t f", kc=NCORES),
        msg1_full[:].rearrange("(kc p) t f -> p kc t f", p=P))

    # ---- PE: phase C ----------------------------------------------
    _psum_zero(t1T, cn["zs128"])
    for q in range(NPAIR):
        j0 = 2 * q
        for h in range(4):
            hs = slice(h * 256, (h + 1) * 256)
            nc.tensor.matmul(
                t1T[:, hs], msg1s[:, j0:j0 + 2, :], mT[:, j0:j0 + 2, hs],
                start=False, stop=(q == NPAIR - 1), perf_mode=DR,
                skip_group_check=True)

    # ---- phase D: batched msg2 (q + residual) ----------------------
    t1sbs = rep_sb.tile([P, NS], BF16, tag="t1sbs", name="t1sbs", bufs=1)
    nc.vector.tensor_mul(t1sbs[:], t1T[:], alpha_bc[:])

    zps_all = psum.tile([P, NS], F32, tag="acc2", name="zps_all")
    _psum_zero(zps_all, cn["zs128"])
    for t in range(G_LOCAL):
        ts = slice(t * P, (t + 1) * P)
        nc.tensor.matmul(zps_all[:, ts], t1sbs[:, ts], cn["w2T"][:],
                         start=False, stop=False, skip_group_check=True)
        nc.tensor.matmul(zps_all[:, ts], scl64_bf[:, ts], cn["b2_row"][:],
                         start=False, stop=True, skip_group_check=True)

    msg2p = rep_sb.tile([P, G_LOCAL, S2, F], FP8, tag="m2p", name="msg2p",
                        bufs=1)
    zview = zps_all[:].rearrange("p (t f) -> p t f", t=G_LOCAL)

    def _t_drelu():
        nc.scalar.activation(msg2p[:, :, 0, :], zview, RELU)

    if not _DEFER:
        _t_drelu()
    if _USE_RESIDUAL:
        # residual err = GE*(relu(z) - q): both operands pre-scaled by GE so
        # the subtract writes the fp8 err stream directly
        m2ball = rep_sb.tile([P, NS], BF16, tag="m1b", name="m2ball", bufs=2)
        nc.scalar.activation(m2ball[:], zps_all[:], RELU, scale=GE)
        m2dall = rep_sb.tile([P, NS], BF16, tag="t1sbs", name="m2dall",
                             bufs=1)
        nc.vector.tensor_scalar_mul(
            m2dall[:].rearrange("p (t f) -> p t f", t=G_LOCAL),
            msg2p[:, :, 0, :], GE)
        nc.vector.tensor_sub(
            msg2p[:, :, 1, :],
            m2ball[:].rearrange("p (t f) -> p t f", t=G_LOCAL),
            m2dall[:].rearrange("p (t f) -> p t f", t=G_LOCAL))

    # ---- rep r's tail: msg2 AG + readback, phase E, phase F --------
    # All emitted inside rep r+1's streams (or the epilogue) so every read
    # follows its producer in emission order.
    msg2_loc_d = dram.tile([P, G_LOCAL, S2, F], FP8, tag="m2ld",
                           name="msg2_loc_d")
    msg2_full = dram.tile([NCORES * P, G_LOCAL, S2, F], FP8, tag="m2f",
                          name="msg2_full", addr_space="Shared")
    m2full = rep_sb.tile([P, JT, S2, F], FP8, tag="m2full", name="m2full",
                         bufs=1)
    if _USE_RESIDUAL:
        t2s = rep_sb.tile([P, NS], F32, tag="t2hs", name="t2s", bufs=2)
        h2a = rep_sb.tile([P, NS], F32, tag="t2hs", name="h2a", bufs=2)
    h2s = rep_sb.tile([P, NS], F32, tag="t2hs", name="h2s", bufs=2)
    pooledT = rep_sb.tile([P, G_LOCAL], F32, tag="pooledT", name="pooledT")
    out_sb = rep_sb.tile([G_LOCAL, C], F32, tag="out_sb", name="out_sb",
                         bufs=3)
    box = {}

    def _t_pool_head():
        nc.gpsimd.dma_start(msg2_loc_d[:], msg2p[:])
        if _USE_COLLECTIVES:
            nc.gpsimd.collective_compute(
                "AllGather", mybir.AluOpType.bypass,
                replica_groups=[list(range(NCORES))],
                ins=[msg2_loc_d.opt()], outs=[msg2_full.opt()],
            )
        else:
            scr2 = dram.tile([NCORES * P, G_LOCAL, S2, F], FP8, tag="m2scr",
                             name="m2scr")
            nc.gpsimd.dma_start(scr2[0:P, :, :, :], msg2_loc_d[:])
            nc.gpsimd.dma_start(msg2_full[:], scr2[:])
        nc.gpsimd.dma_start(
            m2full[:].rearrange("p (kc t) s f -> p kc t s f", kc=NCORES),
            msg2_full[:].rearrange("(kc p) t s f -> p kc t s f", p=P))

    def _t_e():
        t2a = psum.tile([P, NS], F32, tag="acc2", name="t2a")
        box["t2a"] = t2a
        _psum_zero(t2a, cn["zs128"])
        if _USE_RESIDUAL:
            t2b = psum.tile([P, NS], F32, tag="acc1", name="t2b")
            box["t2b"] = t2b
            _psum_zero(t2b, cn["zs128"])
        for q in range(NPAIR):
            j0 = 2 * q
            for h in range(4):
                hs = slice(h * 256, (h + 1) * 256)
                nc.tensor.matmul(
                    t2a[:, hs], m2full[:, j0:j0 + 2, 0, :],
                    mT[:, j0:j0 + 2, hs],
                    start=False, stop=(q == NPAIR - 1), perf_mode=DR,
                    skip_group_check=True)
            if _USE_RESIDUAL:
                for h in range(4):
                    hs = slice(h * 256, (h + 1) * 256)
                    nc.tensor.matmul(
                        box["t2b"][:, hs], m2full[:, j0:j0 + 2, 1, :],
                        mT[:, j0:j0 + 2, hs],
                        start=False, stop=(q == NPAIR - 1), perf_mode=DR,
                        skip_group_check=True)

    def _t_f1():
        if _USE_RESIDUAL:
            nc.vector.tensor_scalar_mul(t2s[:], box["t2b"][:], 1.0 / GE)
            nc.vector.tensor_add(h2a[:], t2s[:], box["t2a"][:])
        else:
            # single stream: h2s = t2a * bcR directly on DVE (one PSUM input)
            nc.vector.tensor_mul(h2s[:], box["t2a"][:], bcR_sb[:])

    def _t_f2():
        if _USE_RESIDUAL:
            nc.gpsimd.tensor_mul(h2s[:], h2a[:], bcR_sb[:])

    def _t_f3():
        for g in range(G_LOCAL):
            nc.vector.reduce_max(out=pooledT[:, g:g + 1],
                                 in_=h2s[:, g * P:(g + 1) * P],
                                 axis=mybir.AxisListType.X)

    def _t_cls():
        cps = ab_psum.tile([G_LOCAL, C], F32, tag="ab", name="cps")
        box["cps"] = cps
        nc.tensor.matmul(cps[:], pooledT[:], cn["wcT"][:],
                         start=True, stop=False)
        nc.tensor.matmul(cps[:], cn["ones8_f32"][:], cn["bc_row"][:],
                         start=False, stop=True)

    def _t_f5():
        nc.vector.tensor_copy(out_sb[:], box["cps"][:])

    def _sp_out():
        nc.sync.dma_start(out_l.ap(), out_sb[:])

    if _DEFER:
        nl["tail"] = [_t_drelu, _t_pool_head, _t_e, _t_f1, _t_f2, _t_f3,
                      _t_cls, _t_f5]
        nl["sp_out"].append(_sp_out)
    else:
        for f in (_t_pool_head, _t_e, _t_f1, _t_f2, _t_f3, _t_cls, _t_f5,
                  _sp_out):
            f()
    return nl


def _get_nc():
    if "nc" not in _CACHE:
        _CACHE["nc"] = _build()
    return _CACHE["nc"]


def _prep_in_maps(inputs):
    m = np.asarray(inputs["m"], dtype=np.float32)
    x = np.asarray(inputs["x"]).astype(np.int32)
    emb = np.asarray(inputs["emb"], dtype=np.float32).astype(NP_BF16)
    w1 = np.ascontiguousarray(np.asarray(inputs["w1"], dtype=np.float32))
    b1 = np.ascontiguousarray(np.asarray(inputs["b1"], dtype=np.float32))
    w2 = np.ascontiguousarray(np.asarray(inputs["w2"], dtype=np.float32))
    b2 = np.ascontiguousarray(np.asarray(inputs["b2"], dtype=np.float32))
    wc = np.ascontiguousarray(np.asarray(inputs["wc"], dtype=np.float32))
    bc = np.ascontiguousarray(np.asarray(inputs["bc"], dtype=np.float32))

    in_maps = []
    for k in range(NCORES):
        # mT_pre[p, jt, i] = m[k*NS + i, jt*P + p], cast to fp8e4m3
        shard = m[k * NS:(k + 1) * NS, :]                      # [i, j]
        mt = np.ascontiguousarray(
            shard.T.reshape(JT, P, NS).transpose(1, 0, 2)).astype(NP_FP8)
        # x_loc[p, t] = x[k*NS + t*128 + p]
        xl = np.ascontiguousarray(
            x[k * NS:(k + 1) * NS].reshape(G_LOCAL, P).T)
        in_maps.append({
            "mT_pre": mt, "x_loc": xl, "emb_in": emb,
            "w1_in": w1, "b1_in": b1, "w2_in": w2, "b2_in": b2,
            "wc_in": wc, "bc_in": bc,
        })
    return in_maps


def kernel(**inputs):
    nc = _get_nc()
    in_maps = _prep_in_maps(inputs)
    res = bass_utils.run_bass_kernel_spmd(
        nc, in_maps, core_ids=list(range(NCORES)))
    out = np.concatenate([res.results[k]["out_l"] for k in range(NCORES)], axis=0)
    return out.astype(np.float32)
